# revision 33
# baseline (speedup 1.0000x reference)
"""Trainium2 Bass kernel for nn_DistributedAttention (dense_transformer).

Computation per position i (fully position-local):
  xn = LayerNorm(x_i); q,k,v = xn@W* + b*  (reshaped to (H=16, D=64))
  scores = q @ k^T / sqrt(D)   -> (16,16) attention over HEADS
  probs  = softmax(scores); att = probs @ v       (16,64)
  out    = att @ Wo + bo                          (16,1024)
  + 3 scalar metrics per position.

Sharding: batch*seq (4096 positions) split evenly across 8 cores; all
weights replicated; no collectives.

Device layout ("mapping A"): features on partitions, positions on the free
axis.  q/k/v live as q.T [head*64+d (8 chunks of 128 partitions), 512 pos].
Per-position HxH attention is done with head-"shift" elementwise products
(DVE) + segmented partition reductions / accumulations on the TensorE via
constant mask matmuls.  Output and most intermediates are bf16; PSUM
accumulation is fp32.
"""

import numpy as np
import ml_dtypes

bf16 = ml_dtypes.bfloat16

B, S, HID = 2, 2048, 1024
H, D = 16, 64
NCORES = 8
POS = (B * S) // NCORES  # 512 positions per core
PCH = POS // 128         # 4 position chunks
KSUB = HID // 128        # 8 contraction subtiles
JC = HID // 128          # 8 feature chunks
HC = H // 2              # 8 head chunks (2 heads per 128-partition chunk)
EPS_LN = 1e-6

_CACHE = {}


# ----------------------------------------------------------------------------
# host-side constant masks
# ----------------------------------------------------------------------------
def _make_masks():
    ident = np.eye(128, dtype=np.float32)
    hp1 = np.zeros((128, HC * 16), dtype=np.float32)
    for c in range(HC):
        for p in range(2):
            hp1[64 * p : 64 * (p + 1), 16 * c + 2 * c + p] = 1.0
    dm = np.zeros((128, 64), dtype=np.float32)
    for p in range(2):
        for d in range(64):
            dm[64 * p + d, d] = 1.0 / 16.0
    cn = np.full((128, 1), -1.0 / 16.0, dtype=np.float32)
    h2 = np.zeros((128, 1), dtype=np.float32); h2[:16] = 1.0 / 16.0
    d2 = np.zeros((128, 1), dtype=np.float32); d2[:64] = 1.0 / 64.0
    bd = np.zeros((128, 128), dtype=np.float32)
    bd[:64, :64] = 1.0
    bd[64:, 64:] = 1.0
    big = np.concatenate([ident, hp1, hp1 * 64.0, dm, cn, h2, d2, bd], axis=1)
    return np.ascontiguousarray(big.astype(bf16))


# ----------------------------------------------------------------------------
# bass program
# ----------------------------------------------------------------------------
def _build_program(use_bo, debug=False):
    import concourse.bass as bass
    import concourse.bacc as bacc
    import concourse.mybir as mybir
    import concourse.tile as tile
    from contextlib import ExitStack

    dt = mybir.dt
    Alu = mybir.AluOpType
    Act = mybir.ActivationFunctionType
    AxX = mybir.AxisListType.X

    nc = bacc.Bacc("TRN2", target_bir_lowering=False, debug=False,
                   enable_asserts=False, num_devices=NCORES)

    f32 = dt.float32
    bf = dt.bfloat16

    # ---- dram io ----
    x_d = nc.dram_tensor("x", [POS, HID], f32, kind="ExternalInput").ap()
    w_d = {t: nc.dram_tensor(f"w{t}", [JC, 128, KSUB, 128], bf, kind="ExternalInput").ap()
           for t in "qkv"}
    wo_d = nc.dram_tensor("wo", [128, HID], bf, kind="ExternalInput").ap()
    bias_d = nc.dram_tensor("bias_qkv", [128, JC, 3], f32, kind="ExternalInput").ap()
    if use_bo:
        bo_d = nc.dram_tensor("bo_row", [1, HID], bf, kind="ExternalInput").ap()
        ones1_d = nc.dram_tensor("ones_1x128", [1, 128], bf, kind="ExternalInput").ap()
    NMASK = 128 + HC * 16 * 2 + 64 + 3 + 128
    mask_d = nc.dram_tensor("masks", [128, NMASK], bf, kind="ExternalInput").ap()

    out_d = nc.dram_tensor("out", [POS * H, HID], bf, kind="ExternalOutput").ap()
    if debug:
        dbg = {n: nc.dram_tensor(f"dbg_{n}", shp, bf, kind="ExternalOutput").ap()
               for n, shp in [("yT", [128, KSUB, POS]), ("q", [128, JC, POS]),
                              ("k", [128, JC, POS]), ("v", [128, JC, POS]),
                              ("krot", [128, JC, POS]),
                              ("att", [128, HC, POS])]}
    ent_d = nc.dram_tensor("ent", [1, POS], f32, kind="ExternalOutput").ap()
    hd_d = nc.dram_tensor("hd", [1, POS], f32, kind="ExternalOutput").ap()
    gn_d = nc.dram_tensor("gn", [1, POS], f32, kind="ExternalOutput").ap()

    with tile.TileContext(nc) as tc, ExitStack() as ctx:
        pool = ctx.enter_context(tc.tile_pool(name="sb", bufs=1))
        xpool = ctx.enter_context(tc.tile_pool(name="xp", bufs=2))
        wpool = ctx.enter_context(tc.tile_pool(name="wp", bufs=3))
        prodp = ctx.enter_context(tc.tile_pool(name="pr", bufs=4))
        outp = ctx.enter_context(tc.tile_pool(name="op", bufs=2))
        psum = ctx.enter_context(tc.tile_pool(name="ps", bufs=2, space="PSUM"))
        psum_sc = ctx.enter_context(tc.tile_pool(name="psS", bufs=2, space="PSUM"))
        psum_att = ctx.enter_context(tc.tile_pool(name="psA", bufs=4, space="PSUM"))

        # ---- load constants ----
        mask_sb = pool.tile([128, NMASK], bf, name="mask_sb")
        nc.sync.dma_start(mask_sb[:], mask_d)
        o_hp1, o_hp64 = 128, 128 + HC * 16
        o_dm = 128 + 2 * HC * 16
        o_cn, o_h2, o_d2 = o_dm + 64, o_dm + 65, o_dm + 66
        masks = {
            "hpair1": mask_sb[:, o_hp1 : o_hp1 + HC * 16].rearrange(
                "p (c t) -> p c t", c=HC),
            "hpair64": mask_sb[:, o_hp64 : o_hp64 + HC * 16].rearrange(
                "p (c t) -> p c t", c=HC),
            "dmean": mask_sb[:, o_dm : o_dm + 64],
            "colneg16": mask_sb[:, o_cn : o_cn + 1],
            "h2one": mask_sb[:16, o_h2 : o_h2 + 1],
            "d2one": mask_sb[:64, o_d2 : o_d2 + 1],
            "bd64": mask_sb[:, o_d2 + 1 : o_d2 + 129],
        }
        ident = mask_sb[:, 0:128]

        wo_sb = pool.tile([128, HID], bf, name="wo_sb")
        nc.sync.dma_start(wo_sb[:], wo_d)
        bias_sb = pool.tile([128, JC, 3], f32, name="bias_sb")
        nc.sync.dma_start(bias_sb[:], bias_d)
        if use_bo:
            bo_sb = pool.tile([1, HID], bf, name="bo_sb")
            nc.sync.dma_start(bo_sb[:], bo_d)
            ones1_sb = pool.tile([1, 128], bf, name="ones1_sb")
            nc.sync.dma_start(ones1_sb[:], ones1_d)

        # ---- phase 1: layernorm + transpose -> yT [128, KSUB, POS] bf16 ----
        yT = pool.tile([128, KSUB, POS], bf, name="yT")
        for ic in range(PCH):
            x_sb = xpool.tile([128, HID], f32, name="x_sb", tag="x")
            nc.sync.dma_start(x_sb[:], x_d[128 * ic : 128 * (ic + 1), :])
            sq_scr = xpool.tile([128, HID], bf, name="sq_scr", tag="sqs")
            ssq = xpool.tile([128, 1], f32, name="ssq", tag="st0")
            nc.scalar.activation(sq_scr[:], x_sb[:], Act.Square, accum_out=ssq[:])
            ssum = xpool.tile([128, 1], f32, name="ssum", tag="st1")
            nc.vector.tensor_reduce(ssum[:], x_sb[:], AxX, Alu.add)
            negmu = xpool.tile([128, 1], f32, name="negmu", tag="st2")
            nc.vector.tensor_scalar_mul(negmu[:], ssum[:], -1.0 / HID)
            musq = xpool.tile([128, 1], f32, name="musq", tag="st3")
            nc.vector.tensor_tensor(musq[:], negmu[:], negmu[:], Alu.mult)
            var = xpool.tile([128, 1], f32, name="var", tag="st4")
            nc.vector.tensor_scalar_mul(var[:], ssq[:], 1.0 / HID)
            nc.vector.tensor_tensor(var[:], var[:], musq[:], Alu.subtract)
            nc.vector.tensor_scalar_add(var[:], var[:], EPS_LN)
            std = xpool.tile([128, 1], f32, name="std", tag="st5")
            nc.scalar.activation(std[:], var[:], Act.Sqrt)
            rstd = xpool.tile([128, 1], f32, name="rstd", tag="st6")
            nc.vector.reciprocal(rstd[:], std[:])
            nmr = xpool.tile([128, 1], f32, name="nmr", tag="st7")
            nc.vector.tensor_tensor(nmr[:], negmu[:], rstd[:], Alu.mult)
            y_bf = xpool.tile([128, HID], bf, name="y_bf", tag="y")
            nc.scalar.activation(y_bf[:], x_sb[:], Act.Identity,
                                 bias=nmr[:], scale=rstd[:])
            for j in range(KSUB):
                tp = psum.tile([128, 128], bf, name=f"tp_{ic}_{j}", tag="bank")
                nc.tensor.transpose(tp[:], y_bf[:, 128 * j : 128 * (j + 1)], ident)
                nc.scalar.copy(yT[:, j, 128 * ic : 128 * (ic + 1)], tp[:])

        if debug:
            nc.sync.dma_start(dbg["yT"], yT[:])
        # ---- phase 2: q/k/v projections -> [128, HC, POS] bf16 (transposed) ----
        qkv_sb = {}
        for ti, t in enumerate("qkv"):
            qkv_sb[t] = pool.tile([128, JC, POS], bf, name=f"{t}_sb")
        for ti, t in enumerate("qkv"):
            for jc in range(JC):
                w_t = wpool.tile([128, KSUB, 128], bf, name=f"w_{t}{jc}", tag="w")
                nc.sync.dma_start(w_t[:], w_d[t][jc])
                ps = psum.tile([128, POS], f32, name=f"pj_{t}{jc}", tag="bank")
                for ks in range(KSUB):
                    nc.tensor.matmul(ps[:], w_t[:, ks, :], yT[:, ks, :],
                                     start=(ks == 0), stop=(ks == KSUB - 1))
                nc.scalar.activation(qkv_sb[t][:, jc, :], ps[:], Act.Identity,
                                     bias=bias_sb[:, jc, ti:ti + 1])

        # ---- phase 2b: partition-rotated copies of k and v (shift by 64) ----
        rot_sb = {}
        for t in "kv":
            src = qkv_sb[t]
            r = pool.tile([128, JC + 3, POS], bf, name=f"{t}rot_sb")
            nc.gpsimd.dma_start(r[0:64, 0:JC, :], src[64:128, :, :])
            nc.gpsimd.dma_start(r[64:128, 0 : JC - 1, :], src[0:64, 1:JC, :])
            nc.gpsimd.dma_start(r[64:128, JC - 1, :], src[0:64, 0, :])
            nc.gpsimd.dma_start(r[:, JC : JC + 3, :], r[:, 0:3, :])
            rot_sb[t] = r
        ext_sb = {}
        for t in "kv":
            e = pool.tile([128, JC + 3, POS], bf, name=f"{t}ext_sb")
            nc.sync.dma_start(e[:, 0:JC, :], qkv_sb[t][:])
            nc.sync.dma_start(e[:, JC : JC + 3, :], qkv_sb[t][:, 0:3, :])
            ext_sb[t] = e

        if debug:
            for t in "qkv":
                nc.sync.dma_start(dbg[t], qkv_sb[t][:])
            nc.sync.dma_start(dbg["krot"], rot_sb["k"][:])

        def shifted(t, s):
            """source tile + chunk offset for head-shift s of tensor t."""
            if s % 2 == 0:
                return ext_sb[t], s // 2
            return rot_sb[t], (s - 1) // 2

        # gradient_norm = mean_h sqrt(64*sum_d q'^2 + sum_d k^2 + sum_d v^2)
        gn_ps = psum.tile([16, POS], f32, name="gn_ps", tag="bank")
        i = 0
        for t, mk in (("q", "hpair64"), ("k", "hpair1"), ("v", "hpair1")):
            for hf in range(2):
                cs = 4 * hf
                sqt = prodp.tile([128, 4, POS], bf, name=f"sq_{t}{hf}", tag="prod")
                nc.vector.tensor_tensor(sqt[:], qkv_sb[t][:, cs : cs + 4, :],
                                        qkv_sb[t][:, cs : cs + 4, :], Alu.mult)
                for ci in range(4):
                    nc.tensor.matmul(gn_ps[:], masks[mk][:, cs + ci, :],
                                     sqt[:, ci, :],
                                     start=(i == 0), stop=(i == 23))
                    i += 1
        gn_sb = pool.tile([16, POS], bf, name="gn_sb")
        nc.scalar.activation(gn_sb[:], gn_ps[:], Act.Sqrt)
        gnm_ps = psum.tile([1, POS], f32, name="gnm_ps", tag="bank")
        nc.tensor.matmul(gnm_ps[:], masks["h2one"], gn_sb[:], start=True, stop=True)
        gnm_sb = pool.tile([1, POS], f32, name="gnm_sb")
        nc.vector.tensor_copy(gnm_sb[:], gnm_ps[:])
        nc.sync.dma_start(gn_d, gnm_sb[:])


        out_view = out_d.rearrange("(i h) j -> i h j", h=H)
        # ---- phases 3-6: attention middle, replicated-scores design.
        # Per chunk-half (4 chunks) and shift s:
        #   prod = q (*) k_shift            (DVE, bf16)
        #   scores_rep = bd64.T @ prod      (PE; scores replicated over d)
        #   exp_rep = Exp(scores_rep)       (ACT, psum->sbuf bf16)
        #   esum += exp_rep                 (DVE even s / GpSimd odd s)
        #   pav = exp_rep (*) v_shift       (DVE)
        #   att_raw[c] += pav[c]            (PE identity matmul, psum accum)
        # att = att_raw * (1/esum) is folded into the psum evacuation.
        att_sb = pool.tile([128, HC, POS], bf, name="att_sb")

        def half_products(out_t, in0_t, t, s, cs, in0_base):
            src2, co = shifted(t, s)
            lo = (cs + co) % HC  # extended tiles: lo+4 <= JC+3 always
            nc.vector.tensor_tensor(out_t[:],
                                    in0_t[:, in0_base : in0_base + 4, :],
                                    src2[:, lo : lo + 4, :], Alu.mult)

        for half in range(2):
            cs = 4 * half
            att_ps = [psum_att.tile([128, POS], f32, name=f"att{half}_{ci}", tag="abank")
                      for ci in range(4)]
            esA = [pool.tile([128, 4, POS], bf, name=f"esA{half}_{k}", tag=f"esA{k}")
                   for k in range(2)]
            esB = [pool.tile([128, 4, POS], bf, name=f"esB{half}_{k}", tag=f"esB{k}")
                   for k in range(2)]
            es_state = {"A": 0, "B": 0, "An": 0, "Bn": 0}
            def emit_scores(s):
                prod = prodp.tile([128, 4, POS], bf, name=f"sp{half}_{s}",
                                  tag="prod")
                half_products(prod, qkv_sb["q"], "k", s, cs, cs)
                exp_rep = prodp.tile([128, 4, POS], bf, name=f"ex{half}_{s}",
                                     tag="exp")
                for ci in range(4):
                    scps = psum_sc.tile([128, POS], f32, name=f"sc{half}_{s}_{ci}",
                                     tag="sbank")
                    nc.tensor.matmul(scps[:], masks["bd64"], prod[:, ci, :],
                                     start=True, stop=True)
                    nc.scalar.activation(exp_rep[:, ci, :], scps[:], Act.Exp)
                return exp_rep

            def emit_av(s, exp_rep):
                pav = prodp.tile([128, 4, POS], bf, name=f"ap{half}_{s}", tag="pav")
                half_products(pav, exp_rep, "v", s, cs, 0)
                for ci in range(4):
                    nc.tensor.matmul(att_ps[ci][:], ident, pav[:, ci, :],
                                     start=(s == 0), stop=(s == H - 1),
                                     skip_group_check=True)
                ch = "A" if s % 2 == 0 else "B"
                eng = nc.vector if ch == "A" else nc.gpsimd
                bufs_ch = esA if ch == "A" else esB
                cur = es_state[ch]
                if es_state[ch + "n"] == 0:
                    nc.vector.tensor_copy(bufs_ch[cur][:], exp_rep[:])
                else:
                    nxt = 1 - cur
                    eng.tensor_tensor(bufs_ch[nxt][:], bufs_ch[cur][:],
                                      exp_rep[:], Alu.add)
                    es_state[ch] = nxt
                es_state[ch + "n"] += 1

            pending = None
            for s in range(H):
                exp_rep = emit_scores(s)
                if pending is not None:
                    emit_av(*pending)
                pending = (s, exp_rep)
            emit_av(*pending)
            esum_f = pool.tile([128, 4, POS], f32, name=f"esf{half}", tag="esf")
            nc.vector.tensor_tensor(esum_f[:], esA[es_state["A"]][:],
                                    esB[es_state["B"]][:], Alu.add)
            recip = pool.tile([128, 4, POS], f32, name=f"recip{half}", tag="rcp")
            with nc.allow_low_precision(reason="softmax denom approx within tol"):
                nc.vector.reciprocal_approx_fast(recip[:], esum_f[:])
            for ci in range(4):
                nc.vector.tensor_tensor(att_sb[:, cs + ci, :], att_ps[ci][:],
                                        recip[:, ci, :], Alu.mult)

            # output projection for this half's heads (overlaps the other half)
            for hl in range(8):
                h = 8 * half + hl
                c, p = h // 2, h % 2
                for ib in range(PCH):
                    lhs = att_sb[64 * p : 64 * (p + 1), c,
                                 128 * ib : 128 * (ib + 1)]
                    rhs = wo_sb[64 * p : 64 * (p + 1), :]
                    o_sb = outp.tile([128, HID], bf, name=f"o_{h}_{ib}", tag="o")
                    for jh in range(2):
                        ps_o = psum.tile([128, HID // 2], f32,
                                         name=f"po_{h}_{ib}_{jh}", tag="bank")
                        nc.tensor.matmul(
                            ps_o[:], lhs, rhs[:, 512 * jh : 512 * (jh + 1)],
                            start=True, stop=not use_bo)
                        if use_bo:
                            nc.tensor.matmul(
                                ps_o[:], ones1_sb[:],
                                bo_sb[:, 512 * jh : 512 * (jh + 1)],
                                start=False, stop=True, skip_group_check=True)
                        dst = o_sb[:, 512 * jh : 512 * (jh + 1)]
                        if (h + ib) % 2 == 0:
                            nc.vector.tensor_copy(dst, ps_o[:])
                        else:
                            nc.scalar.copy(dst, ps_o[:])
                    eng = nc.gpsimd if (h * PCH + ib) % 2 else nc.sync
                    eng.dma_start(
                        out_view[128 * ib : 128 * (ib + 1), h, :], o_sb[:])

        if debug:
            nc.sync.dma_start(dbg["att"], att_sb[:])
        # ---- phase 8: metrics ----
        # entropy = -(1/16) * sum_{h,d} att * log(att + 1e-10)
        c1e10 = pool.tile([128, 1], f32, name="c1e10")
        nc.gpsimd.memset(c1e10[:], 1e-10)
        ent_ps = psum.tile([1, POS], f32, name="ent_ps", tag="bank")
        for hf in range(2):
            cs = 4 * hf
            lg = prodp.tile([128, 4, POS], bf, name=f"lg{hf}", tag="prod")
            nc.scalar.activation(lg[:], att_sb[:, cs : cs + 4, :], Act.Ln,
                                 bias=c1e10[:])
            entp = prodp.tile([128, 4, POS], bf, name=f"entp{hf}", tag="pav")
            nc.vector.tensor_tensor(entp[:], att_sb[:, cs : cs + 4, :], lg[:],
                                    Alu.mult)
            for ci in range(4):
                nc.tensor.matmul(ent_ps[:], masks["colneg16"], entp[:, ci, :],
                                 start=(hf == 0 and ci == 0),
                                 stop=(hf == 1 and ci == 3))
        ent_sb = pool.tile([1, POS], f32, name="ent_sb")
        nc.vector.tensor_copy(ent_sb[:], ent_ps[:])
        nc.sync.dma_start(ent_d, ent_sb[:])

        # head_diversity = mean_d std_h att
        mean_ps = psum.tile([64, POS], f32, name="mean_ps", tag="bank")
        m2_ps = psum.tile([64, POS], f32, name="m2_ps", tag="bank")
        for c in range(HC):
            nc.tensor.matmul(mean_ps[:], masks["dmean"], att_sb[:, c, :],
                             start=(c == 0), stop=(c == HC - 1))
        for hf in range(2):
            cs = 4 * hf
            asq = prodp.tile([128, 4, POS], bf, name=f"asq{hf}", tag="prod")
            nc.vector.tensor_tensor(asq[:], att_sb[:, cs : cs + 4, :],
                                    att_sb[:, cs : cs + 4, :], Alu.mult)
            for ci in range(4):
                nc.tensor.matmul(m2_ps[:], masks["dmean"], asq[:, ci, :],
                                 start=(hf == 0 and ci == 0),
                                 stop=(hf == 1 and ci == 3))
        mean_sb = pool.tile([64, POS], f32, name="mean_sb")
        nc.scalar.copy(mean_sb[:], mean_ps[:])
        var_sb = pool.tile([64, POS], f32, name="var_sb")
        # var = m2 - mean^2 ; clamp at 0
        msq_sb = pool.tile([64, POS], f32, name="msq_sb")
        nc.vector.tensor_tensor(msq_sb[:], mean_sb[:], mean_sb[:], Alu.mult)
        nc.vector.tensor_tensor(var_sb[:], m2_ps[:], msq_sb[:], Alu.subtract)
        nc.vector.tensor_scalar_max(var_sb[:], var_sb[:], 0.0)
        std_sb = pool.tile([64, POS], bf, name="std_sb")
        nc.scalar.activation(std_sb[:], var_sb[:], Act.Sqrt)
        hd_ps = psum.tile([1, POS], f32, name="hd_ps", tag="bank")
        nc.tensor.matmul(hd_ps[:], masks["d2one"], std_sb[:], start=True, stop=True)
        hd_sb = pool.tile([1, POS], f32, name="hd_sb")
        nc.vector.tensor_copy(hd_sb[:], hd_ps[:])
        nc.sync.dma_start(hd_d, hd_sb[:])

    nc.compile()
    return nc


def _get_program(use_bo):
    key = ("prog", use_bo)
    if key not in _CACHE:
        _CACHE[key] = _build_program(use_bo)
    return _CACHE[key]


# ----------------------------------------------------------------------------
# host wrapper
# ----------------------------------------------------------------------------
def _prep_inputs(x, norm_scale, norm_bias, Wq, bq, Wk, bk, Wv, bv, Wo, bo):
    """Fold layernorm affine + 1/sqrt(D) into weights; build per-core in_maps."""
    s = norm_scale.astype(np.float32)
    b = norm_bias.astype(np.float32)

    def fold(W, bias, scale):
        We = (s[:, None] * W.astype(np.float32)) * scale
        be = (b @ W.astype(np.float32) + bias.astype(np.float32)) * scale
        # [jc, p, ks, j] so a per-jc slice is contiguous per partition
        wt = We.reshape(KSUB, 128, JC, 128).transpose(2, 1, 0, 3).astype(bf16)
        return np.ascontiguousarray(wt), be

    wq_t, bq_e = fold(Wq, bq, 0.125)
    wk_t, bk_e = fold(Wk, bk, 1.0)
    wv_t, bv_e = fold(Wv, bv, 1.0)
    bias_qkv = np.stack(
        [be.reshape(JC, 128).T for be in (bq_e, bk_e, bv_e)], axis=2
    ).astype(np.float32)  # [128, JC, 3]
    bias_qkv = np.ascontiguousarray(bias_qkv)

    use_bo = bool(np.any(bo != 0))

    common = {"wq": wq_t, "wk": wk_t, "wv": wv_t,
              "wo": np.ascontiguousarray(np.tile(Wo.astype(bf16), (2, 1))),
              "bias_qkv": bias_qkv, "masks": _make_masks()}
    if use_bo:
        common["bo_row"] = np.ascontiguousarray(bo.astype(bf16)[None, :])
        common["ones_1x128"] = np.ones((1, 128), dtype=bf16)

    xf = np.asarray(x, dtype=np.float32).reshape(B * S, HID)
    in_maps = []
    for c in range(NCORES):
        m = dict(common)
        m["x"] = np.ascontiguousarray(xf[c * POS : (c + 1) * POS])
        in_maps.append(m)
    return in_maps, use_bo


def run(x, norm_scale, norm_bias, Wq, bq, Wk, bk, Wv, bv, Wo, bo, **rk):
    from concourse import bass_utils

    in_maps, use_bo = _prep_inputs(x, norm_scale, norm_bias, Wq, bq,
                                   Wk, bk, Wv, bv, Wo, bo)
    nc = _get_program(use_bo)
    res = bass_utils.run_bass_kernel_spmd(nc, in_maps, core_ids=list(range(NCORES)), **rk)

    outs, ents, hds, gns = [], [], [], []
    for r in res.results:
        outs.append(np.asarray(r["out"]).astype(np.float32))
        ents.append(np.asarray(r["ent"]).reshape(POS))
        hds.append(np.asarray(r["hd"]).reshape(POS))
        gns.append(np.asarray(r["gn"]).reshape(POS))
    out = np.concatenate(outs, axis=0).reshape(B, S, H, HID)
    ent = np.concatenate(ents).reshape(B, S)
    hd = np.concatenate(hds).reshape(B, S)
    gn = np.concatenate(gns).reshape(B, S)
    return (out, ent, hd, gn), res


def kernel(**inputs):
    (out, ent, hd, gn), _ = run(**inputs)
    return out, ent, hd, gn


# revision 35
# speedup vs baseline: 1.0004x; 1.0004x over previous
"""Trainium2 Bass kernel for nn_DistributedAttention (dense_transformer).

Computation per position i (fully position-local):
  xn = LayerNorm(x_i); q,k,v = xn@W* + b*  (reshaped to (H=16, D=64))
  scores = q @ k^T / sqrt(D)   -> (16,16) attention over HEADS
  probs  = softmax(scores); att = probs @ v       (16,64)
  out    = att @ Wo + bo                          (16,1024)
  + 3 scalar metrics per position.

Sharding: batch*seq (4096 positions) split evenly across 8 cores; all
weights replicated; no collectives.

Device layout ("mapping A"): features on partitions, positions on the free
axis.  q/k/v live as q.T [head*64+d (8 chunks of 128 partitions), 512 pos].
Per-position HxH attention is done with head-"shift" elementwise products
(DVE) + segmented partition reductions / accumulations on the TensorE via
constant mask matmuls.  Output and most intermediates are bf16; PSUM
accumulation is fp32.
"""

import numpy as np
import ml_dtypes

bf16 = ml_dtypes.bfloat16

B, S, HID = 2, 2048, 1024
H, D = 16, 64
NCORES = 8
POS = (B * S) // NCORES  # 512 positions per core
PCH = POS // 128         # 4 position chunks
KSUB = HID // 128        # 8 contraction subtiles
JC = HID // 128          # 8 feature chunks
HC = H // 2              # 8 head chunks (2 heads per 128-partition chunk)
EPS_LN = 1e-6

_CACHE = {}


# ----------------------------------------------------------------------------
# host-side constant masks
# ----------------------------------------------------------------------------
def _make_masks():
    ident = np.eye(128, dtype=np.float32)
    hp1 = np.zeros((128, HC * 16), dtype=np.float32)
    for c in range(HC):
        for p in range(2):
            hp1[64 * p : 64 * (p + 1), 16 * c + 2 * c + p] = 1.0
    dm = np.zeros((128, 64), dtype=np.float32)
    for p in range(2):
        for d in range(64):
            dm[64 * p + d, d] = 1.0 / 16.0
    cn = np.full((128, 1), -1.0 / 16.0, dtype=np.float32)
    h2 = np.zeros((128, 1), dtype=np.float32); h2[:16] = 1.0 / 16.0
    d2 = np.zeros((128, 1), dtype=np.float32); d2[:64] = 1.0 / 64.0
    bd = np.zeros((128, 128), dtype=np.float32)
    bd[:64, :64] = 1.0
    bd[64:, 64:] = 1.0
    big = np.concatenate([ident, hp1, hp1 * 64.0, dm, cn, h2, d2, bd], axis=1)
    return np.ascontiguousarray(big.astype(bf16))


# ----------------------------------------------------------------------------
# bass program
# ----------------------------------------------------------------------------
def _build_program(use_bo, debug=False):
    import concourse.bass as bass
    import concourse.bacc as bacc
    import concourse.mybir as mybir
    import concourse.tile as tile
    from contextlib import ExitStack

    dt = mybir.dt
    Alu = mybir.AluOpType
    Act = mybir.ActivationFunctionType
    AxX = mybir.AxisListType.X

    nc = bacc.Bacc("TRN2", target_bir_lowering=False, debug=False,
                   enable_asserts=False, num_devices=NCORES)

    f32 = dt.float32
    bf = dt.bfloat16

    # ---- dram io ----
    x_d = nc.dram_tensor("x", [POS, HID], f32, kind="ExternalInput").ap()
    w_d = {t: nc.dram_tensor(f"w{t}", [JC, 128, KSUB, 128], bf, kind="ExternalInput").ap()
           for t in "qkv"}
    wo_d = nc.dram_tensor("wo", [128, HID], bf, kind="ExternalInput").ap()
    bias_d = nc.dram_tensor("bias_qkv", [128, JC, 3], f32, kind="ExternalInput").ap()
    if use_bo:
        bo_d = nc.dram_tensor("bo_row", [1, HID], bf, kind="ExternalInput").ap()
        ones1_d = nc.dram_tensor("ones_1x128", [1, 128], bf, kind="ExternalInput").ap()
    NMASK = 128 + HC * 16 * 2 + 64 + 3 + 128
    mask_d = nc.dram_tensor("masks", [128, NMASK], bf, kind="ExternalInput").ap()

    out_d = nc.dram_tensor("out", [POS * H, HID], bf, kind="ExternalOutput").ap()
    if debug:
        dbg = {n: nc.dram_tensor(f"dbg_{n}", shp, bf, kind="ExternalOutput").ap()
               for n, shp in [("yT", [128, KSUB, POS]), ("q", [128, JC, POS]),
                              ("k", [128, JC, POS]), ("v", [128, JC, POS]),
                              ("krot", [128, JC, POS]),
                              ("att", [128, HC, POS])]}
    ent_d = nc.dram_tensor("ent", [1, POS], f32, kind="ExternalOutput").ap()
    hd_d = nc.dram_tensor("hd", [1, POS], f32, kind="ExternalOutput").ap()
    gn_d = nc.dram_tensor("gn", [1, POS], f32, kind="ExternalOutput").ap()

    with tile.TileContext(nc) as tc, ExitStack() as ctx:
        pool = ctx.enter_context(tc.tile_pool(name="sb", bufs=1))
        xpool = ctx.enter_context(tc.tile_pool(name="xp", bufs=2))
        wpool = ctx.enter_context(tc.tile_pool(name="wp", bufs=3))
        prodp = ctx.enter_context(tc.tile_pool(name="pr", bufs=4))
        outp = ctx.enter_context(tc.tile_pool(name="op", bufs=2))
        psum = ctx.enter_context(tc.tile_pool(name="ps", bufs=2, space="PSUM"))
        psum_sc = ctx.enter_context(tc.tile_pool(name="psS", bufs=2, space="PSUM"))
        psum_att = ctx.enter_context(tc.tile_pool(name="psA", bufs=4, space="PSUM"))

        # ---- load constants ----
        mask_sb = pool.tile([128, NMASK], bf, name="mask_sb")
        nc.sync.dma_start(mask_sb[:], mask_d)
        o_hp1, o_hp64 = 128, 128 + HC * 16
        o_dm = 128 + 2 * HC * 16
        o_cn, o_h2, o_d2 = o_dm + 64, o_dm + 65, o_dm + 66
        masks = {
            "hpair1": mask_sb[:, o_hp1 : o_hp1 + HC * 16].rearrange(
                "p (c t) -> p c t", c=HC),
            "hpair64": mask_sb[:, o_hp64 : o_hp64 + HC * 16].rearrange(
                "p (c t) -> p c t", c=HC),
            "dmean": mask_sb[:, o_dm : o_dm + 64],
            "colneg16": mask_sb[:, o_cn : o_cn + 1],
            "h2one": mask_sb[:16, o_h2 : o_h2 + 1],
            "d2one": mask_sb[:64, o_d2 : o_d2 + 1],
            "bd64": mask_sb[:, o_d2 + 1 : o_d2 + 129],
        }
        ident = mask_sb[:, 0:128]

        wo_sb = pool.tile([128, HID], bf, name="wo_sb")
        nc.sync.dma_start(wo_sb[:], wo_d)
        bias_sb = pool.tile([128, JC, 3], f32, name="bias_sb")
        nc.sync.dma_start(bias_sb[:], bias_d)
        if use_bo:
            bo_sb = pool.tile([1, HID], bf, name="bo_sb")
            nc.sync.dma_start(bo_sb[:], bo_d)
            ones1_sb = pool.tile([1, 128], bf, name="ones1_sb")
            nc.sync.dma_start(ones1_sb[:], ones1_d)

        # ---- phase 1: layernorm + transpose -> yT [128, KSUB, POS] bf16 ----
        yT = pool.tile([128, KSUB, POS], bf, name="yT")
        for ic in range(PCH):
            x_sb = xpool.tile([128, HID], f32, name="x_sb", tag="x")
            nc.sync.dma_start(x_sb[:], x_d[128 * ic : 128 * (ic + 1), :])
            sq_scr = xpool.tile([128, HID], bf, name="sq_scr", tag="sqs")
            ssq = xpool.tile([128, 1], f32, name="ssq", tag="st0")
            nc.scalar.activation(sq_scr[:], x_sb[:], Act.Square, accum_out=ssq[:])
            ssum = xpool.tile([128, 1], f32, name="ssum", tag="st1")
            nc.vector.tensor_reduce(ssum[:], x_sb[:], AxX, Alu.add)
            negmu = xpool.tile([128, 1], f32, name="negmu", tag="st2")
            nc.vector.tensor_scalar_mul(negmu[:], ssum[:], -1.0 / HID)
            musq = xpool.tile([128, 1], f32, name="musq", tag="st3")
            nc.vector.tensor_tensor(musq[:], negmu[:], negmu[:], Alu.mult)
            var = xpool.tile([128, 1], f32, name="var", tag="st4")
            nc.vector.tensor_scalar_mul(var[:], ssq[:], 1.0 / HID)
            nc.vector.tensor_tensor(var[:], var[:], musq[:], Alu.subtract)
            nc.vector.tensor_scalar_add(var[:], var[:], EPS_LN)
            std = xpool.tile([128, 1], f32, name="std", tag="st5")
            nc.scalar.activation(std[:], var[:], Act.Sqrt)
            rstd = xpool.tile([128, 1], f32, name="rstd", tag="st6")
            nc.vector.reciprocal(rstd[:], std[:])
            nmr = xpool.tile([128, 1], f32, name="nmr", tag="st7")
            nc.vector.tensor_tensor(nmr[:], negmu[:], rstd[:], Alu.mult)
            y_bf = xpool.tile([128, HID], bf, name="y_bf", tag="y")
            nc.scalar.activation(y_bf[:], x_sb[:], Act.Identity,
                                 bias=nmr[:], scale=rstd[:])
            for j in range(KSUB):
                tp = psum.tile([128, 128], bf, name=f"tp_{ic}_{j}", tag="bank")
                nc.tensor.transpose(tp[:], y_bf[:, 128 * j : 128 * (j + 1)], ident)
                nc.scalar.copy(yT[:, j, 128 * ic : 128 * (ic + 1)], tp[:])

        if debug:
            nc.sync.dma_start(dbg["yT"], yT[:])
        # ---- phase 2: q/k/v projections -> [128, HC, POS] bf16 (transposed) ----
        qkv_sb = {"q": pool.tile([128, JC, POS], bf, name="q_sb")}
        ext_sb = {t: pool.tile([128, JC + 3, POS], bf, name=f"{t}ext_sb")
                  for t in "kv"}
        for t in "kv":
            qkv_sb[t] = ext_sb[t][:, 0:JC, :]
        for ti, t in enumerate("qkv"):
            for jc in range(JC):
                w_t = wpool.tile([128, KSUB, 128], bf, name=f"w_{t}{jc}", tag="w")
                nc.sync.dma_start(w_t[:], w_d[t][jc])
                ps = psum.tile([128, POS], f32, name=f"pj_{t}{jc}", tag="bank")
                for ks in range(KSUB):
                    nc.tensor.matmul(ps[:], w_t[:, ks, :], yT[:, ks, :],
                                     start=(ks == 0), stop=(ks == KSUB - 1))
                nc.scalar.activation(qkv_sb[t][:, jc, :], ps[:], Act.Identity,
                                     bias=bias_sb[:, jc, ti:ti + 1])

        # ---- phase 2b: partition-rotated copies of k and v (shift by 64) ----
        rot_sb = {}
        for t in "kv":
            src = qkv_sb[t]
            r = pool.tile([128, JC + 3, POS], bf, name=f"{t}rot_sb")
            nc.gpsimd.dma_start(r[0:64, 0:JC, :], src[64:128, :, :])
            nc.gpsimd.dma_start(r[64:128, 0 : JC - 1, :], src[0:64, 1:JC, :])
            nc.gpsimd.dma_start(r[64:128, JC - 1, :], src[0:64, 0, :])
            nc.gpsimd.dma_start(r[:, JC : JC + 3, :], r[:, 0:3, :])
            rot_sb[t] = r
        for t in "kv":
            nc.sync.dma_start(ext_sb[t][:, JC : JC + 3, :],
                              ext_sb[t][:, 0:3, :])

        if debug:
            for t in "qkv":
                nc.sync.dma_start(dbg[t], qkv_sb[t][:])
            nc.sync.dma_start(dbg["krot"], rot_sb["k"][:])

        def shifted(t, s):
            """source tile + chunk offset for head-shift s of tensor t."""
            if s % 2 == 0:
                return ext_sb[t], s // 2
            return rot_sb[t], (s - 1) // 2

        # gradient_norm = mean_h sqrt(64*sum_d q'^2 + sum_d k^2 + sum_d v^2)
        gn_ps = psum.tile([16, POS], f32, name="gn_ps", tag="bank")
        i = 0
        for t, mk in (("q", "hpair64"), ("k", "hpair1"), ("v", "hpair1")):
            for hf in range(2):
                cs = 4 * hf
                sqt = prodp.tile([128, 4, POS], bf, name=f"sq_{t}{hf}", tag="prod")
                nc.vector.tensor_tensor(sqt[:], qkv_sb[t][:, cs : cs + 4, :],
                                        qkv_sb[t][:, cs : cs + 4, :], Alu.mult)
                for ci in range(4):
                    nc.tensor.matmul(gn_ps[:], masks[mk][:, cs + ci, :],
                                     sqt[:, ci, :],
                                     start=(i == 0), stop=(i == 23))
                    i += 1
        gn_sb = pool.tile([16, POS], bf, name="gn_sb")
        nc.scalar.activation(gn_sb[:], gn_ps[:], Act.Sqrt)
        gnm_ps = psum.tile([1, POS], f32, name="gnm_ps", tag="bank")
        nc.tensor.matmul(gnm_ps[:], masks["h2one"], gn_sb[:], start=True, stop=True)
        gnm_sb = pool.tile([1, POS], f32, name="gnm_sb")
        nc.vector.tensor_copy(gnm_sb[:], gnm_ps[:])
        nc.sync.dma_start(gn_d, gnm_sb[:])


        out_view = out_d.rearrange("(i h) j -> i h j", h=H)
        # ---- phases 3-6: attention middle, replicated-scores design.
        # Per chunk-half (4 chunks) and shift s:
        #   prod = q (*) k_shift            (DVE, bf16)
        #   scores_rep = bd64.T @ prod      (PE; scores replicated over d)
        #   exp_rep = Exp(scores_rep)       (ACT, psum->sbuf bf16)
        #   esum += exp_rep                 (DVE even s / GpSimd odd s)
        #   pav = exp_rep (*) v_shift       (DVE)
        #   att_raw[c] += pav[c]            (PE identity matmul, psum accum)
        # att = att_raw * (1/esum) is folded into the psum evacuation.
        att_sb = pool.tile([128, HC, POS], bf, name="att_sb")
        c1e10 = pool.tile([128, 1], f32, name="c1e10")
        nc.gpsimd.memset(c1e10[:], 1e-10)
        ent_half = [pool.tile([1, POS], f32, name=f"enth{k}") for k in range(2)]
        mean_half = [pool.tile([64, POS], f32, name=f"meanh{k}") for k in range(2)]
        m2_half = [pool.tile([64, POS], f32, name=f"m2h{k}") for k in range(2)]

        def half_products(out_t, in0_t, t, s, cs, in0_base):
            src2, co = shifted(t, s)
            lo = (cs + co) % HC  # extended tiles: lo+4 <= JC+3 always
            nc.vector.tensor_tensor(out_t[:],
                                    in0_t[:, in0_base : in0_base + 4, :],
                                    src2[:, lo : lo + 4, :], Alu.mult)

        for half in range(2):
            cs = 4 * half
            att_ps = [psum_att.tile([128, POS], f32, name=f"att{half}_{ci}", tag="abank")
                      for ci in range(4)]
            esA = [pool.tile([128, 4, POS], bf, name=f"esA{half}_{k}", tag=f"esA{k}")
                   for k in range(2)]
            esB = [pool.tile([128, 4, POS], bf, name=f"esB{half}_{k}", tag=f"esB{k}")
                   for k in range(2)]
            es_state = {"A": 0, "B": 0, "An": 0, "Bn": 0}
            def emit_scores(s):
                prod = prodp.tile([128, 4, POS], bf, name=f"sp{half}_{s}",
                                  tag="prod")
                half_products(prod, qkv_sb["q"], "k", s, cs, cs)
                exp_rep = prodp.tile([128, 4, POS], bf, name=f"ex{half}_{s}",
                                     tag="exp")
                for ci in range(4):
                    scps = psum_sc.tile([128, POS], f32, name=f"sc{half}_{s}_{ci}",
                                     tag="sbank")
                    nc.tensor.matmul(scps[:], masks["bd64"], prod[:, ci, :],
                                     start=True, stop=True)
                    nc.scalar.activation(exp_rep[:, ci, :], scps[:], Act.Exp)
                return exp_rep

            def emit_av(s, exp_rep):
                pav = prodp.tile([128, 4, POS], bf, name=f"ap{half}_{s}", tag="pav")
                half_products(pav, exp_rep, "v", s, cs, 0)
                for ci in range(4):
                    nc.tensor.matmul(att_ps[ci][:], ident, pav[:, ci, :],
                                     start=(s == 0), stop=(s == H - 1),
                                     skip_group_check=True)
                ch = "A" if s % 2 == 0 else "B"
                eng = nc.vector if ch == "A" else nc.gpsimd
                bufs_ch = esA if ch == "A" else esB
                cur = es_state[ch]
                if es_state[ch + "n"] == 0:
                    nc.vector.tensor_copy(bufs_ch[cur][:], exp_rep[:])
                else:
                    nxt = 1 - cur
                    eng.tensor_tensor(bufs_ch[nxt][:], bufs_ch[cur][:],
                                      exp_rep[:], Alu.add)
                    es_state[ch] = nxt
                es_state[ch + "n"] += 1

            pending = None
            for s in range(H):
                exp_rep = emit_scores(s)
                if pending is not None:
                    emit_av(*pending)
                pending = (s, exp_rep)
            emit_av(*pending)
            esum_f = pool.tile([128, 4, POS], f32, name=f"esf{half}", tag="esf")
            nc.vector.tensor_tensor(esum_f[:], esA[es_state["A"]][:],
                                    esB[es_state["B"]][:], Alu.add)
            recip = pool.tile([128, 4, POS], f32, name=f"recip{half}", tag="rcp")
            with nc.allow_low_precision(reason="softmax denom approx within tol"):
                nc.vector.reciprocal_approx_fast(recip[:], esum_f[:])
            for ci in range(4):
                nc.vector.tensor_tensor(att_sb[:, cs + ci, :], att_ps[ci][:],
                                        recip[:, ci, :], Alu.mult)

            # per-half metric partials (entropy + head-diversity moments)
            lg = prodp.tile([128, 4, POS], bf, name=f"lg{half}", tag="prod")
            nc.scalar.activation(lg[:], att_sb[:, cs : cs + 4, :], Act.Ln,
                                 bias=c1e10[:])
            entp = prodp.tile([128, 4, POS], bf, name=f"entp{half}", tag="pav")
            nc.vector.tensor_tensor(entp[:], att_sb[:, cs : cs + 4, :], lg[:],
                                    Alu.mult)
            eh_ps = psum.tile([1, POS], f32, name=f"eh_ps{half}", tag="bank")
            for ci in range(4):
                nc.tensor.matmul(eh_ps[:], masks["colneg16"], entp[:, ci, :],
                                 start=(ci == 0), stop=(ci == 3))
            nc.vector.tensor_copy(ent_half[half][:], eh_ps[:])

            asq = prodp.tile([128, 4, POS], bf, name=f"asq{half}", tag="prod")
            nc.vector.tensor_tensor(asq[:], att_sb[:, cs : cs + 4, :],
                                    att_sb[:, cs : cs + 4, :], Alu.mult)
            mh_ps = psum.tile([64, POS], f32, name=f"mh_ps{half}", tag="bank")
            for ci in range(4):
                nc.tensor.matmul(mh_ps[:], masks["dmean"], att_sb[:, cs + ci, :],
                                 start=(ci == 0), stop=(ci == 3))
            nc.vector.tensor_copy(mean_half[half][:], mh_ps[:])
            m2_ps = psum.tile([64, POS], f32, name=f"m2h_ps{half}", tag="bank")
            for ci in range(4):
                nc.tensor.matmul(m2_ps[:], masks["dmean"], asq[:, ci, :],
                                 start=(ci == 0), stop=(ci == 3))
            nc.vector.tensor_copy(m2_half[half][:], m2_ps[:])

            # output projection for this half's heads (overlaps the other half)
            for hl in range(8):
                h = 8 * half + hl
                c, p = h // 2, h % 2
                for ib in range(PCH):
                    lhs = att_sb[64 * p : 64 * (p + 1), c,
                                 128 * ib : 128 * (ib + 1)]
                    rhs = wo_sb[64 * p : 64 * (p + 1), :]
                    o_sb = outp.tile([128, HID], bf, name=f"o_{h}_{ib}", tag="o")
                    for jh in range(2):
                        ps_o = psum.tile([128, HID // 2], f32,
                                         name=f"po_{h}_{ib}_{jh}", tag="bank")
                        nc.tensor.matmul(
                            ps_o[:], lhs, rhs[:, 512 * jh : 512 * (jh + 1)],
                            start=True, stop=not use_bo)
                        if use_bo:
                            nc.tensor.matmul(
                                ps_o[:], ones1_sb[:],
                                bo_sb[:, 512 * jh : 512 * (jh + 1)],
                                start=False, stop=True, skip_group_check=True)
                        dst = o_sb[:, 512 * jh : 512 * (jh + 1)]
                        if (h + ib) % 2 == 0:
                            nc.vector.tensor_copy(dst, ps_o[:])
                        else:
                            nc.scalar.copy(dst, ps_o[:])
                    eng = nc.gpsimd if (h * PCH + ib) % 2 else nc.sync
                    eng.dma_start(
                        out_view[128 * ib : 128 * (ib + 1), h, :], o_sb[:])

        if debug:
            nc.sync.dma_start(dbg["att"], att_sb[:])
        # ---- phase 8: combine per-half metric partials ----
        ent_sb = pool.tile([1, POS], f32, name="ent_sb")
        nc.vector.tensor_tensor(ent_sb[:], ent_half[0][:], ent_half[1][:], Alu.add)
        nc.sync.dma_start(ent_d, ent_sb[:])

        mean_sb = pool.tile([64, POS], f32, name="mean_sb")
        nc.vector.tensor_tensor(mean_sb[:], mean_half[0][:], mean_half[1][:],
                                Alu.add)
        m2_sb = pool.tile([64, POS], f32, name="m2_sb")
        nc.vector.tensor_tensor(m2_sb[:], m2_half[0][:], m2_half[1][:], Alu.add)
        msq_sb = pool.tile([64, POS], f32, name="msq_sb")
        nc.vector.tensor_tensor(msq_sb[:], mean_sb[:], mean_sb[:], Alu.mult)
        var_sb = pool.tile([64, POS], f32, name="var_sb")
        nc.vector.tensor_tensor(var_sb[:], m2_sb[:], msq_sb[:], Alu.subtract)
        nc.vector.tensor_scalar_max(var_sb[:], var_sb[:], 0.0)
        std_sb = pool.tile([64, POS], bf, name="std_sb")
        nc.scalar.activation(std_sb[:], var_sb[:], Act.Sqrt)
        hd_ps = psum.tile([1, POS], f32, name="hd_ps", tag="bank")
        nc.tensor.matmul(hd_ps[:], masks["d2one"], std_sb[:], start=True, stop=True)
        hd_sb = pool.tile([1, POS], f32, name="hd_sb")
        nc.vector.tensor_copy(hd_sb[:], hd_ps[:])
        nc.sync.dma_start(hd_d, hd_sb[:])

    nc.compile()
    return nc


def _get_program(use_bo):
    key = ("prog", use_bo)
    if key not in _CACHE:
        _CACHE[key] = _build_program(use_bo)
    return _CACHE[key]


# ----------------------------------------------------------------------------
# host wrapper
# ----------------------------------------------------------------------------
def _prep_inputs(x, norm_scale, norm_bias, Wq, bq, Wk, bk, Wv, bv, Wo, bo):
    """Fold layernorm affine + 1/sqrt(D) into weights; build per-core in_maps."""
    s = norm_scale.astype(np.float32)
    b = norm_bias.astype(np.float32)

    def fold(W, bias, scale):
        We = (s[:, None] * W.astype(np.float32)) * scale
        be = (b @ W.astype(np.float32) + bias.astype(np.float32)) * scale
        # [jc, p, ks, j] so a per-jc slice is contiguous per partition
        wt = We.reshape(KSUB, 128, JC, 128).transpose(2, 1, 0, 3).astype(bf16)
        return np.ascontiguousarray(wt), be

    wq_t, bq_e = fold(Wq, bq, 0.125)
    wk_t, bk_e = fold(Wk, bk, 1.0)
    wv_t, bv_e = fold(Wv, bv, 1.0)
    bias_qkv = np.stack(
        [be.reshape(JC, 128).T for be in (bq_e, bk_e, bv_e)], axis=2
    ).astype(np.float32)  # [128, JC, 3]
    bias_qkv = np.ascontiguousarray(bias_qkv)

    use_bo = bool(np.any(bo != 0))

    common = {"wq": wq_t, "wk": wk_t, "wv": wv_t,
              "wo": np.ascontiguousarray(np.tile(Wo.astype(bf16), (2, 1))),
              "bias_qkv": bias_qkv, "masks": _make_masks()}
    if use_bo:
        common["bo_row"] = np.ascontiguousarray(bo.astype(bf16)[None, :])
        common["ones_1x128"] = np.ones((1, 128), dtype=bf16)

    xf = np.asarray(x, dtype=np.float32).reshape(B * S, HID)
    in_maps = []
    for c in range(NCORES):
        m = dict(common)
        m["x"] = np.ascontiguousarray(xf[c * POS : (c + 1) * POS])
        in_maps.append(m)
    return in_maps, use_bo


def run(x, norm_scale, norm_bias, Wq, bq, Wk, bk, Wv, bv, Wo, bo, **rk):
    from concourse import bass_utils

    in_maps, use_bo = _prep_inputs(x, norm_scale, norm_bias, Wq, bq,
                                   Wk, bk, Wv, bv, Wo, bo)
    nc = _get_program(use_bo)
    res = bass_utils.run_bass_kernel_spmd(nc, in_maps, core_ids=list(range(NCORES)), **rk)

    outs, ents, hds, gns = [], [], [], []
    for r in res.results:
        outs.append(np.asarray(r["out"]).astype(np.float32))
        ents.append(np.asarray(r["ent"]).reshape(POS))
        hds.append(np.asarray(r["hd"]).reshape(POS))
        gns.append(np.asarray(r["gn"]).reshape(POS))
    out = np.concatenate(outs, axis=0).reshape(B, S, H, HID)
    ent = np.concatenate(ents).reshape(B, S)
    hd = np.concatenate(hds).reshape(B, S)
    gn = np.concatenate(gns).reshape(B, S)
    return (out, ent, hd, gn), res


def kernel(**inputs):
    (out, ent, hd, gn), _ = run(**inputs)
    return out, ent, hd, gn


# revision 36
# speedup vs baseline: 1.0194x; 1.0190x over previous
"""Trainium2 Bass kernel for nn_DistributedAttention (dense_transformer).

Computation per position i (fully position-local):
  xn = LayerNorm(x_i); q,k,v = xn@W* + b*  (reshaped to (H=16, D=64))
  scores = q @ k^T / sqrt(D)   -> (16,16) attention over HEADS
  probs  = softmax(scores); att = probs @ v       (16,64)
  out    = att @ Wo + bo                          (16,1024)
  + 3 scalar metrics per position.

Sharding: batch*seq (4096 positions) split evenly across 8 cores; all
weights replicated; no collectives.

Device layout ("mapping A"): features on partitions, positions on the free
axis.  q/k/v live as q.T [head*64+d (8 chunks of 128 partitions), 512 pos].
Per-position HxH attention is done with head-"shift" elementwise products
(DVE) + segmented partition reductions / accumulations on the TensorE via
constant mask matmuls.  Output and most intermediates are bf16; PSUM
accumulation is fp32.
"""

import numpy as np
import ml_dtypes

bf16 = ml_dtypes.bfloat16

B, S, HID = 2, 2048, 1024
H, D = 16, 64
NCORES = 8
POS = (B * S) // NCORES  # 512 positions per core
PCH = POS // 128         # 4 position chunks
KSUB = HID // 128        # 8 contraction subtiles
JC = HID // 128          # 8 feature chunks
HC = H // 2              # 8 head chunks (2 heads per 128-partition chunk)
EPS_LN = 1e-6

_CACHE = {}


# ----------------------------------------------------------------------------
# host-side constant masks
# ----------------------------------------------------------------------------
def _make_masks():
    ident = np.eye(128, dtype=np.float32)
    hp1 = np.zeros((128, HC * 16), dtype=np.float32)
    for c in range(HC):
        for p in range(2):
            hp1[64 * p : 64 * (p + 1), 16 * c + 2 * c + p] = 1.0
    dm = np.zeros((128, 64), dtype=np.float32)
    for p in range(2):
        for d in range(64):
            dm[64 * p + d, d] = 1.0 / 16.0
    cn = np.full((128, 1), -1.0 / 16.0, dtype=np.float32)
    h2 = np.zeros((128, 1), dtype=np.float32); h2[:16] = 1.0 / 16.0
    d2 = np.zeros((128, 1), dtype=np.float32); d2[:64] = 1.0 / 64.0
    bd = np.zeros((128, 128), dtype=np.float32)
    bd[:64, :64] = 1.0
    bd[64:, 64:] = 1.0
    big = np.concatenate([ident, hp1, hp1 * 64.0, dm, cn, h2, d2, bd], axis=1)
    return np.ascontiguousarray(big.astype(bf16))


# ----------------------------------------------------------------------------
# bass program
# ----------------------------------------------------------------------------
def _build_program(use_bo, debug=False):
    import concourse.bass as bass
    import concourse.bacc as bacc
    import concourse.mybir as mybir
    import concourse.tile as tile
    from contextlib import ExitStack

    dt = mybir.dt
    Alu = mybir.AluOpType
    Act = mybir.ActivationFunctionType
    AxX = mybir.AxisListType.X

    nc = bacc.Bacc("TRN2", target_bir_lowering=False, debug=False,
                   enable_asserts=False, num_devices=NCORES)

    f32 = dt.float32
    bf = dt.bfloat16

    # ---- dram io ----
    x_d = nc.dram_tensor("x", [POS, HID], f32, kind="ExternalInput").ap()
    w_d = {t: nc.dram_tensor(f"w{t}", [JC, 128, KSUB, 128], bf, kind="ExternalInput").ap()
           for t in "qkv"}
    wo_d = nc.dram_tensor("wo", [128, HID], bf, kind="ExternalInput").ap()
    bias_d = nc.dram_tensor("bias_qkv", [128, JC, 3], f32, kind="ExternalInput").ap()
    if use_bo:
        bo_d = nc.dram_tensor("bo_row", [1, HID], bf, kind="ExternalInput").ap()
        ones1_d = nc.dram_tensor("ones_1x128", [1, 128], bf, kind="ExternalInput").ap()
    NMASK = 128 + HC * 16 * 2 + 64 + 3 + 128
    mask_d = nc.dram_tensor("masks", [128, NMASK], bf, kind="ExternalInput").ap()

    out_d = nc.dram_tensor("out", [POS * H, HID], bf, kind="ExternalOutput").ap()
    if debug:
        dbg = {n: nc.dram_tensor(f"dbg_{n}", shp, bf, kind="ExternalOutput").ap()
               for n, shp in [("yT", [128, KSUB, POS]), ("q", [128, JC, POS]),
                              ("k", [128, JC, POS]), ("v", [128, JC, POS]),
                              ("krot", [128, JC, POS]),
                              ("att", [128, HC, POS])]}
    ent_d = nc.dram_tensor("ent", [1, POS], f32, kind="ExternalOutput").ap()
    hd_d = nc.dram_tensor("hd", [1, POS], f32, kind="ExternalOutput").ap()
    gn_d = nc.dram_tensor("gn", [1, POS], f32, kind="ExternalOutput").ap()

    with tile.TileContext(nc) as tc, ExitStack() as ctx:
        pool = ctx.enter_context(tc.tile_pool(name="sb", bufs=1))
        xpool = ctx.enter_context(tc.tile_pool(name="xp", bufs=2))
        wpool = ctx.enter_context(tc.tile_pool(name="wp", bufs=3))
        prodp = ctx.enter_context(tc.tile_pool(name="pr", bufs=4))
        outp = ctx.enter_context(tc.tile_pool(name="op", bufs=2))
        psum = ctx.enter_context(tc.tile_pool(name="ps", bufs=2, space="PSUM"))
        psum_sc = ctx.enter_context(tc.tile_pool(name="psS", bufs=2, space="PSUM"))
        psum_att = ctx.enter_context(tc.tile_pool(name="psA", bufs=4, space="PSUM"))

        # ---- load constants ----
        mask_sb = pool.tile([128, NMASK], bf, name="mask_sb")
        nc.sync.dma_start(mask_sb[:], mask_d)
        o_hp1, o_hp64 = 128, 128 + HC * 16
        o_dm = 128 + 2 * HC * 16
        o_cn, o_h2, o_d2 = o_dm + 64, o_dm + 65, o_dm + 66
        masks = {
            "hpair1": mask_sb[:, o_hp1 : o_hp1 + HC * 16].rearrange(
                "p (c t) -> p c t", c=HC),
            "hpair64": mask_sb[:, o_hp64 : o_hp64 + HC * 16].rearrange(
                "p (c t) -> p c t", c=HC),
            "dmean": mask_sb[:, o_dm : o_dm + 64],
            "colneg16": mask_sb[:, o_cn : o_cn + 1],
            "h2one": mask_sb[:16, o_h2 : o_h2 + 1],
            "d2one": mask_sb[:64, o_d2 : o_d2 + 1],
            "bd64": mask_sb[:, o_d2 + 1 : o_d2 + 129],
        }
        ident = mask_sb[:, 0:128]

        wo_sb = pool.tile([128, HID], bf, name="wo_sb")
        nc.sync.dma_start(wo_sb[:], wo_d)
        bias_sb = pool.tile([128, JC, 3], f32, name="bias_sb")
        nc.sync.dma_start(bias_sb[:], bias_d)
        if use_bo:
            bo_sb = pool.tile([1, HID], bf, name="bo_sb")
            nc.sync.dma_start(bo_sb[:], bo_d)
            ones1_sb = pool.tile([1, 128], bf, name="ones1_sb")
            nc.sync.dma_start(ones1_sb[:], ones1_d)

        # ---- phase 1: layernorm + transpose -> yT [128, KSUB, POS] bf16 ----
        yT = pool.tile([128, KSUB, POS], bf, name="yT")
        for ic in range(PCH):
            x_sb = xpool.tile([128, HID], f32, name="x_sb", tag="x")
            nc.sync.dma_start(x_sb[:], x_d[128 * ic : 128 * (ic + 1), :])
            sq_scr = xpool.tile([128, HID], bf, name="sq_scr", tag="sqs")
            ssq = xpool.tile([128, 1], f32, name="ssq", tag="st0")
            nc.scalar.activation(sq_scr[:], x_sb[:], Act.Square, accum_out=ssq[:])
            ssum = xpool.tile([128, 1], f32, name="ssum", tag="st1")
            nc.vector.tensor_reduce(ssum[:], x_sb[:], AxX, Alu.add)
            negmu = xpool.tile([128, 1], f32, name="negmu", tag="st2")
            nc.vector.tensor_scalar_mul(negmu[:], ssum[:], -1.0 / HID)
            musq = xpool.tile([128, 1], f32, name="musq", tag="st3")
            nc.vector.tensor_tensor(musq[:], negmu[:], negmu[:], Alu.mult)
            var = xpool.tile([128, 1], f32, name="var", tag="st4")
            nc.vector.tensor_scalar_mul(var[:], ssq[:], 1.0 / HID)
            nc.vector.tensor_tensor(var[:], var[:], musq[:], Alu.subtract)
            nc.vector.tensor_scalar_add(var[:], var[:], EPS_LN)
            std = xpool.tile([128, 1], f32, name="std", tag="st5")
            nc.scalar.activation(std[:], var[:], Act.Sqrt)
            rstd = xpool.tile([128, 1], f32, name="rstd", tag="st6")
            nc.vector.reciprocal(rstd[:], std[:])
            nmr = xpool.tile([128, 1], f32, name="nmr", tag="st7")
            nc.vector.tensor_tensor(nmr[:], negmu[:], rstd[:], Alu.mult)
            y_bf = xpool.tile([128, HID], bf, name="y_bf", tag="y")
            nc.scalar.activation(y_bf[:], x_sb[:], Act.Identity,
                                 bias=nmr[:], scale=rstd[:])
            for j in range(KSUB):
                tp = psum.tile([128, 128], bf, name=f"tp_{ic}_{j}", tag="bank")
                nc.tensor.transpose(tp[:], y_bf[:, 128 * j : 128 * (j + 1)], ident)
                nc.scalar.copy(yT[:, j, 128 * ic : 128 * (ic + 1)], tp[:])

        if debug:
            nc.sync.dma_start(dbg["yT"], yT[:])
        # ---- phase 2: q/k/v projections -> [128, HC, POS] bf16 (transposed) ----
        qkv_sb = {"q": pool.tile([128, JC, POS], bf, name="q_sb")}
        ext_sb = {t: pool.tile([128, JC + 3, POS], bf, name=f"{t}ext_sb")
                  for t in "kv"}
        for t in "kv":
            qkv_sb[t] = ext_sb[t][:, 0:JC, :]
        for ti, t in enumerate("qkv"):
            for jc in range(JC):
                w_t = wpool.tile([128, KSUB, 128], bf, name=f"w_{t}{jc}", tag="w")
                nc.sync.dma_start(w_t[:], w_d[t][jc])
                ps = psum.tile([128, POS], f32, name=f"pj_{t}{jc}", tag="bank")
                for ks in range(KSUB):
                    nc.tensor.matmul(ps[:], w_t[:, ks, :], yT[:, ks, :],
                                     start=(ks == 0), stop=(ks == KSUB - 1))
                nc.scalar.activation(qkv_sb[t][:, jc, :], ps[:], Act.Identity,
                                     bias=bias_sb[:, jc, ti:ti + 1])

        # ---- phase 2b: partition-rotated copies of k and v (shift by 64) ----
        rot_sb = {}
        for t in "kv":
            src = qkv_sb[t]
            r = pool.tile([128, JC + 3, POS], bf, name=f"{t}rot_sb")
            nc.gpsimd.dma_start(r[0:64, 0:JC, :], src[64:128, :, :])
            nc.gpsimd.dma_start(r[64:128, 0 : JC - 1, :], src[0:64, 1:JC, :])
            nc.gpsimd.dma_start(r[64:128, JC - 1, :], src[0:64, 0, :])
            nc.gpsimd.dma_start(r[:, JC : JC + 3, :], r[:, 0:3, :])
            rot_sb[t] = r
        for t in "kv":
            nc.sync.dma_start(ext_sb[t][:, JC : JC + 3, :],
                              ext_sb[t][:, 0:3, :])

        if debug:
            for t in "qkv":
                nc.sync.dma_start(dbg[t], qkv_sb[t][:])
            nc.sync.dma_start(dbg["krot"], rot_sb["k"][:])

        def shifted(t, s):
            """source tile + chunk offset for head-shift s of tensor t."""
            if s % 2 == 0:
                return ext_sb[t], s // 2
            return rot_sb[t], (s - 1) // 2

        # gradient_norm = mean_h sqrt(64*sum_d q'^2 + sum_d k^2 + sum_d v^2)
        gn_ps = psum.tile([16, POS], f32, name="gn_ps", tag="bank")
        i = 0
        for t, mk in (("q", "hpair64"), ("k", "hpair1"), ("v", "hpair1")):
            for hf in range(2):
                cs = 4 * hf
                sqt = prodp.tile([128, 4, POS], bf, name=f"sq_{t}{hf}", tag="prod")
                nc.vector.tensor_tensor(sqt[:], qkv_sb[t][:, cs : cs + 4, :],
                                        qkv_sb[t][:, cs : cs + 4, :], Alu.mult)
                for ci in range(4):
                    nc.tensor.matmul(gn_ps[:], masks[mk][:, cs + ci, :],
                                     sqt[:, ci, :],
                                     start=(i == 0), stop=(i == 23))
                    i += 1
        gn_sb = pool.tile([16, POS], bf, name="gn_sb")
        nc.scalar.activation(gn_sb[:], gn_ps[:], Act.Sqrt)
        gnm_ps = psum.tile([1, POS], f32, name="gnm_ps", tag="bank")
        nc.tensor.matmul(gnm_ps[:], masks["h2one"], gn_sb[:], start=True, stop=True)
        gnm_sb = pool.tile([1, POS], f32, name="gnm_sb")
        nc.vector.tensor_copy(gnm_sb[:], gnm_ps[:])
        nc.sync.dma_start(gn_d, gnm_sb[:])


        out_view = out_d.rearrange("(i h) j -> i h j", h=H)
        # ---- phases 3-6: attention middle, replicated-scores design.
        # Per chunk-half (4 chunks) and shift s:
        #   prod = q (*) k_shift            (DVE, bf16)
        #   scores_rep = bd64.T @ prod      (PE; scores replicated over d)
        #   exp_rep = Exp(scores_rep)       (ACT, psum->sbuf bf16)
        #   esum += exp_rep                 (DVE even s / GpSimd odd s)
        #   pav = exp_rep (*) v_shift       (DVE)
        #   att_raw[c] += pav[c]            (PE identity matmul, psum accum)
        # att = att_raw * (1/esum) is folded into the psum evacuation.
        att_sb = pool.tile([128, HC, POS], bf, name="att_sb")
        c1e10 = pool.tile([128, 1], f32, name="c1e10")
        nc.gpsimd.memset(c1e10[:], 1e-10)
        ent_half = [pool.tile([1, POS], f32, name=f"enth{k}") for k in range(2)]
        mean_half = [pool.tile([64, POS], f32, name=f"meanh{k}") for k in range(2)]
        m2_half = [pool.tile([64, POS], f32, name=f"m2h{k}") for k in range(2)]

        def half_products(out_t, in0_t, t, s, cs, in0_base):
            src2, co = shifted(t, s)
            lo = (cs + co) % HC  # extended tiles: lo+4 <= JC+3 always
            nc.vector.tensor_tensor(out_t[:],
                                    in0_t[:, in0_base : in0_base + 4, :],
                                    src2[:, lo : lo + 4, :], Alu.mult)

        for half in range(2):
            cs = 4 * half
            att_ps = [psum_att.tile([128, POS], f32, name=f"att{half}_{ci}", tag="abank")
                      for ci in range(4)]
            esA = [pool.tile([128, 4, POS], bf, name=f"esA{half}_{k}", tag=f"esA{k}")
                   for k in range(2)]
            esB = [pool.tile([128, 4, POS], bf, name=f"esB{half}_{k}", tag=f"esB{k}")
                   for k in range(2)]
            es_state = {"A": 0, "B": 0, "An": 0, "Bn": 0}
            def emit_scores(s):
                prod = prodp.tile([128, 4, POS], bf, name=f"sp{half}_{s}",
                                  tag="prod")
                half_products(prod, qkv_sb["q"], "k", s, cs, cs)
                exp_rep = prodp.tile([128, 4, POS], bf, name=f"ex{half}_{s}",
                                     tag="exp")
                for ci in range(4):
                    scps = psum_sc.tile([128, POS], f32, name=f"sc{half}_{s}_{ci}",
                                     tag="sbank")
                    nc.tensor.matmul(scps[:], masks["bd64"], prod[:, ci, :],
                                     start=True, stop=True)
                    nc.scalar.activation(exp_rep[:, ci, :], scps[:], Act.Exp)
                return exp_rep

            def emit_av(s, exp_rep):
                pav = prodp.tile([128, 4, POS], bf, name=f"ap{half}_{s}", tag="pav")
                half_products(pav, exp_rep, "v", s, cs, 0)
                for ci in range(4):
                    nc.tensor.matmul(att_ps[ci][:], ident, pav[:, ci, :],
                                     start=(s == 0), stop=(s == H - 1),
                                     skip_group_check=True)
                ch = "A" if s % 2 == 0 else "B"
                eng = nc.vector if ch == "A" else nc.gpsimd
                bufs_ch = esA if ch == "A" else esB
                cur = es_state[ch]
                if es_state[ch + "n"] == 0:
                    nc.vector.tensor_copy(bufs_ch[cur][:], exp_rep[:])
                else:
                    nxt = 1 - cur
                    eng.tensor_tensor(bufs_ch[nxt][:], bufs_ch[cur][:],
                                      exp_rep[:], Alu.add)
                    es_state[ch] = nxt
                es_state[ch + "n"] += 1

            pending = None
            for s in range(H):
                exp_rep = emit_scores(s)
                if pending is not None:
                    emit_av(*pending)
                pending = (s, exp_rep)
            emit_av(*pending)
            esum_f = pool.tile([128, 4, POS], f32, name=f"esf{half}", tag="esf")
            nc.vector.tensor_tensor(esum_f[:], esA[es_state["A"]][:],
                                    esB[es_state["B"]][:], Alu.add)
            recip = pool.tile([128, 4, POS], f32, name=f"recip{half}", tag="rcp")
            with nc.allow_low_precision(reason="softmax denom approx within tol"):
                nc.vector.reciprocal_approx_fast(recip[:], esum_f[:])
            for ci in range(4):
                nc.vector.tensor_tensor(att_sb[:, cs + ci, :], att_ps[ci][:],
                                        recip[:, ci, :], Alu.mult)

            # per-half metric partials (entropy + head-diversity moments)
            lg = prodp.tile([128, 4, POS], bf, name=f"lg{half}", tag="prod")
            nc.scalar.activation(lg[:], att_sb[:, cs : cs + 4, :], Act.Ln,
                                 bias=c1e10[:])
            entp = prodp.tile([128, 4, POS], bf, name=f"entp{half}", tag="pav")
            nc.vector.tensor_tensor(entp[:], att_sb[:, cs : cs + 4, :], lg[:],
                                    Alu.mult)
            eh_ps = psum.tile([1, POS], f32, name=f"eh_ps{half}", tag="bank")
            for ci in range(4):
                nc.tensor.matmul(eh_ps[:], masks["colneg16"], entp[:, ci, :],
                                 start=(ci == 0), stop=(ci == 3))
            nc.vector.tensor_copy(ent_half[half][:], eh_ps[:])

            asq = prodp.tile([128, 4, POS], bf, name=f"asq{half}", tag="prod")
            nc.vector.tensor_tensor(asq[:], att_sb[:, cs : cs + 4, :],
                                    att_sb[:, cs : cs + 4, :], Alu.mult)
            mh_ps = psum.tile([64, POS], f32, name=f"mh_ps{half}", tag="bank")
            for ci in range(4):
                nc.tensor.matmul(mh_ps[:], masks["dmean"], att_sb[:, cs + ci, :],
                                 start=(ci == 0), stop=(ci == 3))
            nc.vector.tensor_copy(mean_half[half][:], mh_ps[:])
            m2_ps = psum.tile([64, POS], f32, name=f"m2h_ps{half}", tag="bank")
            for ci in range(4):
                nc.tensor.matmul(m2_ps[:], masks["dmean"], asq[:, ci, :],
                                 start=(ci == 0), stop=(ci == 3))
            nc.vector.tensor_copy(m2_half[half][:], m2_ps[:])

            # output projection for this half's heads (overlaps the other half)
            for hl in range(8):
                h = 8 * half + hl
                c, p = h // 2, h % 2
                for ib in range(PCH):
                    lhs = att_sb[64 * p : 64 * (p + 1), c,
                                 128 * ib : 128 * (ib + 1)]
                    rhs = wo_sb[64 * p : 64 * (p + 1), :]
                    o_sb = outp.tile([128, HID], bf, name=f"o_{h}_{ib}", tag="o")
                    for jh in range(2):
                        ps_o = psum.tile([128, HID // 2], f32,
                                         name=f"po_{h}_{ib}_{jh}", tag="bank")
                        nc.tensor.matmul(
                            ps_o[:], lhs, rhs[:, 512 * jh : 512 * (jh + 1)],
                            start=True, stop=not use_bo)
                        if use_bo:
                            nc.tensor.matmul(
                                ps_o[:], ones1_sb[:],
                                bo_sb[:, 512 * jh : 512 * (jh + 1)],
                                start=False, stop=True, skip_group_check=True)
                        dst = o_sb[:, 512 * jh : 512 * (jh + 1)]
                        if (2 * ib + jh) % 3 == 0:
                            nc.vector.tensor_copy(dst, ps_o[:])
                        else:
                            nc.scalar.copy(dst, ps_o[:])
                    eng = nc.gpsimd if (h * PCH + ib) % 2 else nc.sync
                    eng.dma_start(
                        out_view[128 * ib : 128 * (ib + 1), h, :], o_sb[:])

        if debug:
            nc.sync.dma_start(dbg["att"], att_sb[:])
        # ---- phase 8: combine per-half metric partials ----
        ent_sb = pool.tile([1, POS], f32, name="ent_sb")
        nc.vector.tensor_tensor(ent_sb[:], ent_half[0][:], ent_half[1][:], Alu.add)
        nc.sync.dma_start(ent_d, ent_sb[:])

        mean_sb = pool.tile([64, POS], f32, name="mean_sb")
        nc.vector.tensor_tensor(mean_sb[:], mean_half[0][:], mean_half[1][:],
                                Alu.add)
        m2_sb = pool.tile([64, POS], f32, name="m2_sb")
        nc.vector.tensor_tensor(m2_sb[:], m2_half[0][:], m2_half[1][:], Alu.add)
        msq_sb = pool.tile([64, POS], f32, name="msq_sb")
        nc.vector.tensor_tensor(msq_sb[:], mean_sb[:], mean_sb[:], Alu.mult)
        var_sb = pool.tile([64, POS], f32, name="var_sb")
        nc.vector.tensor_tensor(var_sb[:], m2_sb[:], msq_sb[:], Alu.subtract)
        nc.vector.tensor_scalar_max(var_sb[:], var_sb[:], 0.0)
        std_sb = pool.tile([64, POS], bf, name="std_sb")
        nc.scalar.activation(std_sb[:], var_sb[:], Act.Sqrt)
        hd_ps = psum.tile([1, POS], f32, name="hd_ps", tag="bank")
        nc.tensor.matmul(hd_ps[:], masks["d2one"], std_sb[:], start=True, stop=True)
        hd_sb = pool.tile([1, POS], f32, name="hd_sb")
        nc.vector.tensor_copy(hd_sb[:], hd_ps[:])
        nc.sync.dma_start(hd_d, hd_sb[:])

    nc.compile()
    return nc


def _get_program(use_bo):
    key = ("prog", use_bo)
    if key not in _CACHE:
        _CACHE[key] = _build_program(use_bo)
    return _CACHE[key]


# ----------------------------------------------------------------------------
# host wrapper
# ----------------------------------------------------------------------------
def _prep_inputs(x, norm_scale, norm_bias, Wq, bq, Wk, bk, Wv, bv, Wo, bo):
    """Fold layernorm affine + 1/sqrt(D) into weights; build per-core in_maps."""
    s = norm_scale.astype(np.float32)
    b = norm_bias.astype(np.float32)

    def fold(W, bias, scale):
        We = (s[:, None] * W.astype(np.float32)) * scale
        be = (b @ W.astype(np.float32) + bias.astype(np.float32)) * scale
        # [jc, p, ks, j] so a per-jc slice is contiguous per partition
        wt = We.reshape(KSUB, 128, JC, 128).transpose(2, 1, 0, 3).astype(bf16)
        return np.ascontiguousarray(wt), be

    wq_t, bq_e = fold(Wq, bq, 0.125)
    wk_t, bk_e = fold(Wk, bk, 1.0)
    wv_t, bv_e = fold(Wv, bv, 1.0)
    bias_qkv = np.stack(
        [be.reshape(JC, 128).T for be in (bq_e, bk_e, bv_e)], axis=2
    ).astype(np.float32)  # [128, JC, 3]
    bias_qkv = np.ascontiguousarray(bias_qkv)

    use_bo = bool(np.any(bo != 0))

    common = {"wq": wq_t, "wk": wk_t, "wv": wv_t,
              "wo": np.ascontiguousarray(np.tile(Wo.astype(bf16), (2, 1))),
              "bias_qkv": bias_qkv, "masks": _make_masks()}
    if use_bo:
        common["bo_row"] = np.ascontiguousarray(bo.astype(bf16)[None, :])
        common["ones_1x128"] = np.ones((1, 128), dtype=bf16)

    xf = np.asarray(x, dtype=np.float32).reshape(B * S, HID)
    in_maps = []
    for c in range(NCORES):
        m = dict(common)
        m["x"] = np.ascontiguousarray(xf[c * POS : (c + 1) * POS])
        in_maps.append(m)
    return in_maps, use_bo


def run(x, norm_scale, norm_bias, Wq, bq, Wk, bk, Wv, bv, Wo, bo, **rk):
    from concourse import bass_utils

    in_maps, use_bo = _prep_inputs(x, norm_scale, norm_bias, Wq, bq,
                                   Wk, bk, Wv, bv, Wo, bo)
    nc = _get_program(use_bo)
    res = bass_utils.run_bass_kernel_spmd(nc, in_maps, core_ids=list(range(NCORES)), **rk)

    outs, ents, hds, gns = [], [], [], []
    for r in res.results:
        outs.append(np.asarray(r["out"]).astype(np.float32))
        ents.append(np.asarray(r["ent"]).reshape(POS))
        hds.append(np.asarray(r["hd"]).reshape(POS))
        gns.append(np.asarray(r["gn"]).reshape(POS))
    out = np.concatenate(outs, axis=0).reshape(B, S, H, HID)
    ent = np.concatenate(ents).reshape(B, S)
    hd = np.concatenate(hds).reshape(B, S)
    gn = np.concatenate(gns).reshape(B, S)
    return (out, ent, hd, gn), res


def kernel(**inputs):
    (out, ent, hd, gn), _ = run(**inputs)
    return out, ent, hd, gn
